# revision 18
# baseline (speedup 1.0000x reference)
"""Trainium2 Bass kernel for nn_ConvPair (pairwise-MLP message passing).

Reference computation (N=1024 atoms, F=8 feats, H=128 hidden, O=3 out):
    hi = x @ W1[:F];  hj = x @ W1[F:]
    h  = tanh(hi[:,None,:] + hj[None,:,:] + b1)        # [N,N,H]
    h  = tanh(h @ W2 + b2)                             # [N,N,H]
    y  = tanh(h @ W3 + b3)                             # [N,N,O]
    out = y.sum(axis=(1,2))                            # [N]

Sharding: outer atom dim i split across 8 cores (128 i per core); the small
weights and the full atom table are replicated. No cross-core reduction.

The ACT (scalar) engine is the roofline: 2 tanh per (pair, hidden) element
= 262k columns/core at 1 col/cycle @1.2GHz. To get under that, tanh1 is
SPLIT between ACT and the otherwise-idle DVE:
  - j < AJ:   DVE broadcast-add (bf16, 4x mode), one big ACT tanh per group
  - j >= AJ:  exact tanh addition identity on DVE only, using
              host-precomputed tb = tanh(hj), ta = tanh(hi + b1):
                den = tb*ta_i + 1      (tensor_scalar 2-op, fp32, 2x)
                r   = 1/den            (reciprocal_approx_fast, 1 pass)
                h1  = (tb + ta_i) * r  (scalar_tensor_tensor, bf16 out)
All matmuls run in bf16 (1 cyc/row on PE + fast weight load). Final
accuracy ~3e-3 rel (gate 2e-2).

Per-core device pipeline, hidden-major [H=128 partitions, ...]:
  prep:  DVE  h1 pre-activations/identity-tanh for the NEXT group
  tanh1: ACT  one in-place tanh per group of G=16 i  [128, G*AJ]
  mm1:   PE   W2 stationary (bf16, FWL), h1 moving -> ps1 [128,1024] f32
  tanh2: ACT  tanh(ps1 + b2) -> h2 bf16 SBUF      bias = b2 column
  mm3:   PE   8x (h2 128-pair chunk stationary) @ W3pad -> ps3 [128,4]
  tanh3: ACT  one in-place tanh per group over ps3 [128, G*32]
  red:   DVE  free-axis reduce ps3 -> ACC[:, i]   (j-offset partials)
  final: PE   ACC.T @ ones -> per-i scalars, ACT copy, DMA out.

The PE stream is software-pipelined (mm1 of step k+1 emitted before mm3 of
step k) so the PE ping-pong never blocks ACT; mm3/tanh3/reduce of a group's
last step drain during the next group's big tanh1.

b3 is zeros for this problem (asserted; a numpy fallback handles the
hypothetical nonzero case exactly).

Wait-discipline: walrus's datapath codegen supports only one semaphore
wait per instruction; _legalize_waits hoists extras onto chained NoOps.
"""

import json

import numpy as np
from contextlib import ExitStack

import bass_rust
import concourse.bass as bass
import concourse.tile as tile
from concourse import mybir
from concourse.bass_utils import run_bass_kernel_spmd

f32 = mybir.dt.float32
bf16 = mybir.dt.bfloat16
Tanh = mybir.ActivationFunctionType.Tanh
Alu = mybir.AluOpType

N, F, H, O = 1024, 8, 128, 3
NCORES = 8
IPC = N // NCORES  # 128 atoms (i) per core
NJ = N             # full j dimension on every core
G = 16             # i's per group (tanh1/tanh3 batching)
NCH = NJ // H      # 8 stage-3 chunks of 128 pairs
OPAD = 4           # W3 padded 3 -> 4 cols (aligned psum writes; pad col = 0)
AJ = 544           # j-columns of tanh1 handled by ACT; rest by DVE identity
J2 = NJ - AJ

# fp32-column layout of the packed per-core constant block [H, CCOLS]
_HJB = 0                  # hj.T bf16-packed            [H, NJ/2]
_HIB = _HJB + NJ // 2     # (hi + b1).T fp32            [H, IPC]
_TB = _HIB + IPC          # tanh(hj).T fp32             [H, NJ]
_TA = _TB + NJ            # tanh(hi + b1).T fp32        [H, IPC]
_W2B = _TA + IPC          # W2 bf16-packed              [H, H/2]
_W3B = _W2B + H // 2      # W3pad bf16-packed           [H, OPAD/2]
_B2 = _W3B + OPAD // 2
_ONES = _B2 + 1
CCOLS = _ONES + 1


def _bf16_pack(a):
    """[P, C] f32 -> [P, C//2] f32 whose words hold bf16 pairs (RNE)."""
    u = np.ascontiguousarray(a, np.float32).view(np.uint32)
    r = ((u >> 16) & 1) + 0x7FFF
    h = ((u + r) >> 16).astype(np.uint32)          # bf16 bit patterns
    h = h.reshape(a.shape[0], -1, 2)
    return (h[:, :, 0] | (h[:, :, 1] << 16)).view(np.float32)


# TPB instructions have a single 8-byte events field: 2 sync commands max
# (walrus rejects more).  Queue-engine DMA ops handle their own sync.
_MULTIWAIT_OK = {"DMACopy", "TriggeredCopy", "Call", "ISA"}


def _legalize_waits(nc):
    """Hoist excess semaphore waits from datapath instructions onto chained
    NoOps (one wait each) so every instruction fits walrus's sync budget.
    Mutates the module in place (a JSON round-trip would drop the extra
    fields of InstCustomDveAnt and break codegen)."""
    counter = 0
    for fn in nc.m.functions:
        for blk in fn.blocks:
            insts = blk.instructions
            out = []
            for inst in insts:
                si = inst.sync_info
                waits = list(si.on_wait) if si is not None else []
                if (si is not None and len(waits) > 1
                        and str(inst.opcode) not in _MULTIWAIT_OK):
                    # keep zero waits on the instruction; one NoOp per wait
                    for w in waits:
                        counter += 1
                        out.append(mybir.InstNoOp(
                            name=f"W-hoist-{counter}",
                            engine=inst.engine,
                            debug=inst.debug,
                            ins=[],
                            outs=[],
                            sync_info=mybir.SyncInfo(on_wait=[w],
                                                     on_update=[]),
                        ))
                    si.on_wait = []
                out.append(inst)
            if len(out) != len(insts):
                del insts[:]
                insts.extend(out)
    return counter


def _build(ipc, nj, reps=1, aj=AJ, ablate=()):
    """Build the per-core Bass program (SPMD: same program, per-core data).

    reps > 1 repeats the main loop (recomputing identical results) via a
    hardware For_i and is used only for differential timing.
    ablate: subset of {"mm3","mm1","tanh1","tanh2","dve"} — drops those
    stages (WRONG OUTPUT; timing experiments only)."""
    assert nj % H == 0 and ipc % G == 0
    j2 = nj - aj
    ngrp = ipc // G

    nc = bass.Bass()
    cparam = nc.declare_dram_parameter("c", [H, CCOLS], f32, isOutput=False)
    yparam = nc.declare_dram_parameter("y", [ipc, 1], f32, isOutput=True)

    with tile.TileContext(nc) as tc:
        with ExitStack() as ctx:
            consts = ctx.enter_context(tc.tile_pool(name="consts", bufs=1))
            h1p = ctx.enter_context(tc.tile_pool(name="h1p", bufs=4))
            denp = ctx.enter_context(tc.tile_pool(name="denp", bufs=2))
            h2p = ctx.enter_context(tc.tile_pool(name="h2p", bufs=3))
            scrp = ctx.enter_context(tc.tile_pool(name="scrp", bufs=1))
            accp = ctx.enter_context(tc.tile_pool(name="accp", bufs=1))
            # PSUM budget (8 banks): ps1 3x2 + (ps3/warm/fin pool) 2x1
            psA = ctx.enter_context(tc.tile_pool(name="psA", bufs=3, space="PSUM"))
            psB = ctx.enter_context(tc.tile_pool(name="psB", bufs=2, space="PSUM"))

            C = consts.tile([H, CCOLS], f32)
            nc.sync.dma_start(out=C, in_=cparam[:, :])

            HJB = C[:, _HJB:_HIB].bitcast(bf16)       # [H, nj]    bf16
            HIB = C[:, _HIB:_TB]                      # [H, ipc]   f32
            TB = C[:, _TB:_TA]                        # [H, nj]    f32
            TA = C[:, _TA:_W2B]                       # [H, ipc]   f32
            W2B = C[:, _W2B:_W3B].bitcast(bf16)       # [H, H]     bf16
            W3B = C[:, _W3B:_B2].bitcast(bf16)        # [H, OPAD]  bf16
            B2 = C[:, _B2:_B2 + 1]                    # [H, 1]     f32
            ONES = C[:, _ONES:_ONES + 1]              # [H, 1]     f32
            TBD = TB[:, aj:]                          # DVE-share tanh(hj)

            ACC = accp.tile([H, ipc], f32)            # [j-offset, i] partials
            warm = scrp.tile([H, 1], f32, tag="warm")

            # --- warmups: let ACT, PE and DVE observe the const-DMA
            # semaphore (and load the tanh table) on single-wait
            # instructions, so no in-loop instruction waits on the DMA.
            nc.scalar.activation(out=warm, in_=B2, func=Tanh)
            warm_t = psB.tile([H, G, NCH, OPAD], f32, tag="s3")
            nc.tensor.matmul(warm_t[:1, 0, 0, :1], C[:, _B2:_B2 + 1],
                             C[:, _B2:_B2 + 1], start=True, stop=True)
            warmv = scrp.tile([H, 1], f32, tag="warmv")
            nc.vector.tensor_scalar_add(warmv, B2, 0.0)

            def prep_dve(i, h1s, dens):
                """DVE: exact identity-tanh for slot i's j >= aj columns."""
                h1 = h1p.tile([H, nj], bf16, tag="h1")
                if "dve" in ablate:
                    nc.vector.tensor_scalar_add(
                        h1[:, aj:], HJB[:, aj:], HIB[:, i:i + 1])
                    h1s[i] = h1
                    return
                den = denp.tile([H, j2], f32, tag="den")
                nc.vector.tensor_scalar(
                    den, TBD, TA[:, i:i + 1], 1.0, Alu.mult, Alu.add)
                nc.vector.reciprocal_approx_fast(out=den, in_=den)
                nc.vector.scalar_tensor_tensor(
                    out=h1[:, aj:], in0=TBD, scalar=TA[:, i:i + 1],
                    in1=den, op0=Alu.add, op1=Alu.mult)
                h1s[i], dens[i] = h1, den

            def tanh1(i, h1s):
                """ACT: tanh(hj + hib_i) for slot i's j < aj columns, via the
                free affine bias port."""
                if "tanh1" in ablate:
                    if i == 0:  # h1 ACT-share must be written at least once
                        nc.scalar.activation(out=h1s[i][:, :aj],
                                             in_=HJB[:, :aj], func=Tanh,
                                             bias=HIB[:, i:i + 1])
                    return
                nc.scalar.activation(out=h1s[i][:, :aj], in_=HJB[:, :aj],
                                     func=Tanh, bias=HIB[:, i:i + 1])

            def flush_mm3(p):
                """Emit the deferred stage-3 matmuls for one (group, k)."""
                ps3, k, h2 = p
                for c in range(1 if "mm3" in ablate else NCH):
                    nc.tensor.matmul(
                        ps3[:, k, c, :],
                        h2[:, c * H:(c + 1) * H],
                        W3B,
                        start=True, stop=True,
                    )

            def tanh3(g, ps3):
                nc.scalar.activation(out=ps3[:, :, :, :], in_=ps3[:, :, :, :],
                                     func=Tanh)

            def reduce_group(g, ps3):
                nc.vector.tensor_reduce(
                    out=ACC[:, g * G:(g + 1) * G],
                    in_=ps3.rearrange("p g c o -> p g (c o)"),
                    axis=mybir.AxisListType.X,
                    op=mybir.AluOpType.add,
                )

            def group_pass():
                """Deep software pipeline over 128 slots: at slot s, DVE
                preps slot s+4, ACT tanh1's slot s+2, PE runs mm1(s+1) and
                mm3(s-2), ACT tanh2's slot s.  Every consumer's input is
                ready >=1 slot early, so no engine blocks on another within
                a slot."""
                h1s, dens, h2s, ps3s = {}, {}, {}, {}
                t3q = []              # groups whose last mm3 was emitted
                redq = []             # groups past tanh3, awaiting reduce
                for s in range(-4, ipc + 4):
                    ip = s + 4
                    if 0 <= ip < ipc:
                        prep_dve(ip, h1s, dens)         # DVE
                    if redq:
                        reduce_group(*redq.pop(0))      # DVE, after prep
                    im = s + 1
                    if 0 <= im < ipc:
                        h1 = h1s.pop(im)
                        ps1 = psA.tile([H, nj], f32)
                        if "mm1" not in ablate or im < 3:
                            for t in range(nj // 512):
                                nc.tensor.matmul(
                                    ps1[:, t * 512:(t + 1) * 512],
                                    W2B, h1[:, t * 512:(t + 1) * 512],
                                    start=True, stop=True)
                        h2s[im] = (ps1, None)
                    i3 = s - 2
                    if 0 <= i3 < ipc:
                        g3, k3 = divmod(i3, G)
                        if k3 == 0:
                            ps3_t = psB.tile([H, G, NCH, OPAD], f32,
                                             tag="s3")
                            ps3s[g3] = ps3_t
                        flush_mm3((ps3s[g3], k3, h2s.pop(i3)[1]))   # PE
                        if k3 == G - 1:
                            t3q.append(g3)
                    it1 = s + 2
                    if 0 <= it1 < ipc:
                        tanh1(it1, h1s)                 # ACT
                    if t3q and t3q[0] * G + G - 1 <= s - 3:
                        g = t3q.pop(0)
                        tanh3(g, ps3s[g])               # ACT
                        redq.append((g, ps3s.pop(g)))
                    if 0 <= s < ipc:
                        ps1, _ = h2s[s]
                        h2 = h2p.tile([H, nj], bf16)
                        if "tanh2" not in ablate or s < 4:
                            nc.scalar.activation(out=h2, in_=ps1, func=Tanh,
                                                 bias=B2)   # ACT
                        h2s[s] = (ps1, h2)
                for g, ps3 in [(g, p) for g, p in ps3s.items()]:
                    tanh3(g, ps3)
                    redq.append((g, ps3))
                for ent in redq:
                    reduce_group(*ent)

            if reps == 1:
                group_pass()
            else:
                # hardware loop: identical iterations, used for timing only
                with tc.For_i(0, reps):
                    group_pass()

            # --- reduce over the 128 j-offset partitions: out = ACC.T @ ones
            fin_t = psB.tile([H, G, NCH, OPAD], f32, tag="s3")
            fin = fin_t[:, 0, 0, 0:1]
            nc.tensor.matmul(fin, ACC, ONES, start=True, stop=True)
            yout = scrp.tile([ipc, 1], f32, tag="yout")
            nc.scalar.copy(yout, fin)
            nc.sync.dma_start(out=yparam[:, :], in_=yout)

    # populate .instr bytes for InstISA subclasses (custom-DVE ops); raw
    # Bass skips this Bacc pass and walrus then fails with "ISA wrong length"
    mybir.codegen_inst_isa_subclasses(nc)
    _legalize_waits(nc)
    return nc


_NC_CACHE = {}


def _get_nc(ipc, nj):
    key = (ipc, nj)
    if key not in _NC_CACHE:
        _NC_CACHE[key] = _build(ipc, nj)
    return _NC_CACHE[key]


def make_in_maps(x, W1, b1, W2, b2, W3, b3):
    """Per-core packed const blocks. Returns list of {"c": [H,CCOLS] f32}."""
    x = np.asarray(x, np.float32)
    W1 = np.asarray(W1, np.float32)
    hi = x @ W1[:F]          # [N, H]
    hj = x @ W1[F:]          # [N, H]
    hib = hi + np.asarray(b1, np.float32)[None, :]   # fold b1
    hj_t = np.ascontiguousarray(hj[:NJ].T)           # [H, nj]
    W3pad = np.zeros((H, OPAD), np.float32)
    W3pad[:, :O] = np.asarray(W3, np.float32)

    hjb = _bf16_pack(hj_t)
    tb = np.tanh(hj_t)
    w2b = _bf16_pack(np.asarray(W2, np.float32))
    w3b = _bf16_pack(W3pad)

    in_maps = []
    for c in range(NCORES):
        blk = np.empty((H, CCOLS), np.float32)
        hibc = hib[c * IPC:(c + 1) * IPC].T          # [H, IPC]
        blk[:, _HJB:_HIB] = hjb
        blk[:, _HIB:_TB] = hibc
        blk[:, _TB:_TA] = tb
        blk[:, _TA:_W2B] = np.tanh(hibc)
        blk[:, _W2B:_W3B] = w2b
        blk[:, _W3B:_B2] = w3b
        blk[:, _B2] = np.asarray(b2, np.float32)
        blk[:, _ONES] = 1.0
        in_maps.append({"c": blk})
    return in_maps


def kernel(x, W1, b1, W2, b2, W3, b3):
    b3 = np.asarray(b3, np.float32)
    if np.any(b3 != 0.0):
        # Never hit for this problem (spec fills b3 with zeros); exact
        # numpy fallback keeps the kernel correct for arbitrary inputs.
        return _numpy_ref(np.asarray(x, np.float32), np.asarray(W1, np.float32),
                          np.asarray(b1, np.float32), np.asarray(W2, np.float32),
                          np.asarray(b2, np.float32), np.asarray(W3, np.float32),
                          b3)

    in_maps = make_in_maps(x, W1, b1, W2, b2, W3, b3)
    nc = _get_nc(IPC, NJ)
    res = run_bass_kernel_spmd(nc, in_maps, list(range(NCORES)))
    out = np.concatenate(
        [res.results[c]["y"].reshape(IPC) for c in range(NCORES)]
    ).astype(np.float32)
    return out


def _numpy_ref(x, W1, b1, W2, b2, W3, b3):
    hi = x @ W1[:F]
    hj = x @ W1[F:]
    out = np.empty((N,), np.float32)
    for i in range(N):
        h = np.tanh(hi[i][None, :] + hj + b1[None, :])
        h = np.tanh(h @ W2 + b2[None, :])
        y = np.tanh(h @ W3 + b3[None, :])
        out[i] = y.sum()
    return out


# revision 19
# speedup vs baseline: 1.0570x; 1.0570x over previous
"""Trainium2 Bass kernel for nn_ConvPair (pairwise-MLP message passing).

Reference computation (N=1024 atoms, F=8 feats, H=128 hidden, O=3 out):
    hi = x @ W1[:F];  hj = x @ W1[F:]
    h  = tanh(hi[:,None,:] + hj[None,:,:] + b1)        # [N,N,H]
    h  = tanh(h @ W2 + b2)                             # [N,N,H]
    y  = tanh(h @ W3 + b3)                             # [N,N,O]
    out = y.sum(axis=(1,2))                            # [N]

Sharding: outer atom dim i split across 8 cores (128 i per core); the small
weights and the full atom table are replicated. No cross-core reduction.

The ACT (scalar) engine is the roofline: 2 tanh per (pair, hidden) element
= 262k columns/core at 1 col/cycle @1.2GHz. To get under that, tanh1 is
SPLIT between ACT and the otherwise-idle DVE:
  - j < AJ:   DVE broadcast-add (bf16, 4x mode), one big ACT tanh per group
  - j >= AJ:  exact tanh addition identity on DVE only, using
              host-precomputed tb = tanh(hj), ta = tanh(hi + b1):
                den = tb*ta_i + 1      (tensor_scalar 2-op, fp32, 2x)
                r   = 1/den            (reciprocal_approx_fast, 1 pass)
                h1  = (tb + ta_i) * r  (scalar_tensor_tensor, bf16 out)
All matmuls run in bf16 (1 cyc/row on PE + fast weight load). Final
accuracy ~3e-3 rel (gate 2e-2).

Per-core device pipeline, hidden-major [H=128 partitions, ...]:
  prep:  DVE  h1 pre-activations/identity-tanh for the NEXT group
  tanh1: ACT  one in-place tanh per group of G=16 i  [128, G*AJ]
  mm1:   PE   W2 stationary (bf16, FWL), h1 moving -> ps1 [128,1024] f32
  tanh2: ACT  tanh(ps1 + b2) -> h2 bf16 SBUF      bias = b2 column
  mm3:   PE   8x (h2 128-pair chunk stationary) @ W3pad -> ps3 [128,4]
  tanh3: ACT  one in-place tanh per group over ps3 [128, G*32]
  red:   DVE  free-axis reduce ps3 -> ACC[:, i]   (j-offset partials)
  final: PE   ACC.T @ ones -> per-i scalars, ACT copy, DMA out.

The PE stream is software-pipelined (mm1 of step k+1 emitted before mm3 of
step k) so the PE ping-pong never blocks ACT; mm3/tanh3/reduce of a group's
last step drain during the next group's big tanh1.

b3 is zeros for this problem (asserted; a numpy fallback handles the
hypothetical nonzero case exactly).

Wait-discipline: walrus's datapath codegen supports only one semaphore
wait per instruction; _legalize_waits hoists extras onto chained NoOps.
"""

import json

import numpy as np
from contextlib import ExitStack

import bass_rust
import concourse.bass as bass
import concourse.tile as tile
from concourse import mybir
from concourse.bass_utils import run_bass_kernel_spmd

f32 = mybir.dt.float32
bf16 = mybir.dt.bfloat16
Tanh = mybir.ActivationFunctionType.Tanh
Alu = mybir.AluOpType

N, F, H, O = 1024, 8, 128, 3
NCORES = 8
IPC = N // NCORES  # 128 atoms (i) per core
NJ = N             # full j dimension on every core
G = 16             # i's per group (tanh1/tanh3 batching)
NCH = NJ // H      # 8 stage-3 chunks of 128 pairs
OPAD = 4           # W3 padded 3 -> 4 cols (aligned psum writes; pad col = 0)
AJ = 608           # j-columns of tanh1 handled by ACT; rest by DVE identity
J2 = NJ - AJ

# fp32-column layout of the packed per-core constant block [H, CCOLS]
_HJB = 0                  # hj.T bf16-packed            [H, NJ/2]
_HIB = _HJB + NJ // 2     # (hi + b1).T fp32            [H, IPC]
_TB = _HIB + IPC          # tanh(hj).T fp32             [H, NJ]
_TA = _TB + NJ            # tanh(hi + b1).T fp32        [H, IPC]
_W2B = _TA + IPC          # W2 bf16-packed              [H, H/2]
_W3B = _W2B + H // 2      # W3pad bf16-packed           [H, OPAD/2]
_B2 = _W3B + OPAD // 2
_ONES = _B2 + 1
CCOLS = _ONES + 1


def _bf16_pack(a):
    """[P, C] f32 -> [P, C//2] f32 whose words hold bf16 pairs (RNE)."""
    u = np.ascontiguousarray(a, np.float32).view(np.uint32)
    r = ((u >> 16) & 1) + 0x7FFF
    h = ((u + r) >> 16).astype(np.uint32)          # bf16 bit patterns
    h = h.reshape(a.shape[0], -1, 2)
    return (h[:, :, 0] | (h[:, :, 1] << 16)).view(np.float32)


# TPB instructions have a single 8-byte events field: 2 sync commands max
# (walrus rejects more).  Queue-engine DMA ops handle their own sync.
_MULTIWAIT_OK = {"DMACopy", "TriggeredCopy", "Call", "ISA"}


def _legalize_waits(nc):
    """Hoist excess semaphore waits from datapath instructions onto chained
    NoOps (one wait each) so every instruction fits walrus's sync budget.
    Mutates the module in place (a JSON round-trip would drop the extra
    fields of InstCustomDveAnt and break codegen)."""
    counter = 0
    for fn in nc.m.functions:
        for blk in fn.blocks:
            insts = blk.instructions
            out = []
            for inst in insts:
                si = inst.sync_info
                waits = list(si.on_wait) if si is not None else []
                if (si is not None and len(waits) > 1
                        and str(inst.opcode) not in _MULTIWAIT_OK):
                    # keep zero waits on the instruction; one NoOp per wait
                    for w in waits:
                        counter += 1
                        out.append(mybir.InstNoOp(
                            name=f"W-hoist-{counter}",
                            engine=inst.engine,
                            debug=inst.debug,
                            ins=[],
                            outs=[],
                            sync_info=mybir.SyncInfo(on_wait=[w],
                                                     on_update=[]),
                        ))
                    si.on_wait = []
                out.append(inst)
            if len(out) != len(insts):
                del insts[:]
                insts.extend(out)
    return counter


def _build(ipc, nj, reps=1, aj=AJ, ablate=()):
    """Build the per-core Bass program (SPMD: same program, per-core data).

    reps > 1 repeats the main loop (recomputing identical results) via a
    hardware For_i and is used only for differential timing.
    ablate: subset of {"mm3","mm1","tanh1","tanh2","dve"} — drops those
    stages (WRONG OUTPUT; timing experiments only)."""
    assert nj % H == 0 and ipc % G == 0
    j2 = nj - aj
    ngrp = ipc // G

    nc = bass.Bass()
    cparam = nc.declare_dram_parameter("c", [H, CCOLS], f32, isOutput=False)
    yparam = nc.declare_dram_parameter("y", [ipc, 1], f32, isOutput=True)

    with tile.TileContext(nc) as tc:
        with ExitStack() as ctx:
            consts = ctx.enter_context(tc.tile_pool(name="consts", bufs=1))
            h1p = ctx.enter_context(tc.tile_pool(name="h1p", bufs=4))
            denp = ctx.enter_context(tc.tile_pool(name="denp", bufs=2))
            h2p = ctx.enter_context(tc.tile_pool(name="h2p", bufs=3))
            scrp = ctx.enter_context(tc.tile_pool(name="scrp", bufs=1))
            accp = ctx.enter_context(tc.tile_pool(name="accp", bufs=1))
            # PSUM budget (8 banks): ps1 3x2 + (ps3/warm/fin pool) 2x1
            psA = ctx.enter_context(tc.tile_pool(name="psA", bufs=3, space="PSUM"))
            psB = ctx.enter_context(tc.tile_pool(name="psB", bufs=2, space="PSUM"))

            C = consts.tile([H, CCOLS], f32)
            nc.sync.dma_start(out=C, in_=cparam[:, :])

            HJB = C[:, _HJB:_HIB].bitcast(bf16)       # [H, nj]    bf16
            HIB = C[:, _HIB:_TB]                      # [H, ipc]   f32
            TB = C[:, _TB:_TA]                        # [H, nj]    f32
            TA = C[:, _TA:_W2B]                       # [H, ipc]   f32
            W2B = C[:, _W2B:_W3B].bitcast(bf16)       # [H, H]     bf16
            W3B = C[:, _W3B:_B2].bitcast(bf16)        # [H, OPAD]  bf16
            B2 = C[:, _B2:_B2 + 1]                    # [H, 1]     f32
            ONES = C[:, _ONES:_ONES + 1]              # [H, 1]     f32
            TBD = TB[:, aj:]                          # DVE-share tanh(hj)

            ACC = accp.tile([H, ipc], f32)            # [j-offset, i] partials
            warm = scrp.tile([H, 1], f32, tag="warm")

            # --- warmups: let ACT, PE and DVE observe the const-DMA
            # semaphore (and load the tanh table) on single-wait
            # instructions, so no in-loop instruction waits on the DMA.
            nc.scalar.activation(out=warm, in_=B2, func=Tanh)
            warm_t = psB.tile([H, G, NCH, OPAD], f32, tag="s3")
            nc.tensor.matmul(warm_t[:1, 0, 0, :1], C[:, _B2:_B2 + 1],
                             C[:, _B2:_B2 + 1], start=True, stop=True)
            warmv = scrp.tile([H, 1], f32, tag="warmv")
            nc.vector.tensor_scalar_add(warmv, B2, 0.0)

            def prep_dve(i, h1s, dens):
                """DVE: exact identity-tanh for slot i's j >= aj columns."""
                h1 = h1p.tile([H, nj], bf16, tag="h1")
                if "dve" in ablate:
                    nc.vector.tensor_scalar_add(
                        h1[:, aj:], HJB[:, aj:], HIB[:, i:i + 1])
                    h1s[i] = h1
                    return
                den = denp.tile([H, j2], f32, tag="den")
                nc.vector.tensor_scalar(
                    den, TBD, TA[:, i:i + 1], 1.0, Alu.mult, Alu.add)
                nc.vector.reciprocal_approx_fast(out=den, in_=den)
                nc.vector.scalar_tensor_tensor(
                    out=h1[:, aj:], in0=TBD, scalar=TA[:, i:i + 1],
                    in1=den, op0=Alu.add, op1=Alu.mult)
                h1s[i], dens[i] = h1, den

            def tanh1(i, h1s):
                """ACT: tanh(hj + hib_i) for slot i's j < aj columns, via the
                free affine bias port."""
                if "tanh1" in ablate:
                    if i == 0:  # h1 ACT-share must be written at least once
                        nc.scalar.activation(out=h1s[i][:, :aj],
                                             in_=HJB[:, :aj], func=Tanh,
                                             bias=HIB[:, i:i + 1])
                    return
                nc.scalar.activation(out=h1s[i][:, :aj], in_=HJB[:, :aj],
                                     func=Tanh, bias=HIB[:, i:i + 1])

            def flush_mm3(p):
                """Emit the deferred stage-3 matmuls for one (group, k)."""
                ps3, k, h2 = p
                for c in range(1 if "mm3" in ablate else NCH):
                    nc.tensor.matmul(
                        ps3[:, k, c, :],
                        h2[:, c * H:(c + 1) * H],
                        W3B,
                        start=True, stop=True,
                    )

            def tanh3(g, ps3):
                nc.scalar.activation(out=ps3[:, :, :, :], in_=ps3[:, :, :, :],
                                     func=Tanh)

            def reduce_group(g, ps3):
                nc.vector.tensor_reduce(
                    out=ACC[:, g * G:(g + 1) * G],
                    in_=ps3.rearrange("p g c o -> p g (c o)"),
                    axis=mybir.AxisListType.X,
                    op=mybir.AluOpType.add,
                )

            def group_pass():
                """Deep software pipeline over 128 slots: at slot s, DVE
                preps slot s+4, ACT tanh1's slot s+2, PE runs mm1(s+1) and
                mm3(s-2), ACT tanh2's slot s.  Every consumer's input is
                ready >=1 slot early, so no engine blocks on another within
                a slot."""
                h1s, dens, h2s, ps3s = {}, {}, {}, {}
                t3q = []              # groups whose last mm3 was emitted
                redq = []             # groups past tanh3, awaiting reduce
                for s in range(-4, ipc + 4):
                    ip = s + 4
                    if 0 <= ip < ipc:
                        prep_dve(ip, h1s, dens)         # DVE
                    if redq:
                        reduce_group(*redq.pop(0))      # DVE, after prep
                    im = s + 1
                    if 0 <= im < ipc:
                        h1 = h1s.pop(im)
                        ps1 = psA.tile([H, nj], f32)
                        if "mm1" not in ablate or im < 3:
                            for t in range(nj // 512):
                                nc.tensor.matmul(
                                    ps1[:, t * 512:(t + 1) * 512],
                                    W2B, h1[:, t * 512:(t + 1) * 512],
                                    start=True, stop=True)
                        h2s[im] = (ps1, None)
                    i3 = s - 2
                    if 0 <= i3 < ipc:
                        g3, k3 = divmod(i3, G)
                        if k3 == 0:
                            ps3_t = psB.tile([H, G, NCH, OPAD], f32,
                                             tag="s3")
                            ps3s[g3] = ps3_t
                        flush_mm3((ps3s[g3], k3, h2s.pop(i3)[1]))   # PE
                        if k3 == G - 1:
                            t3q.append(g3)
                    it1 = s + 2
                    if 0 <= it1 < ipc:
                        tanh1(it1, h1s)                 # ACT
                    if t3q and t3q[0] * G + G - 1 <= s - 3:
                        g = t3q.pop(0)
                        tanh3(g, ps3s[g])               # ACT
                        redq.append((g, ps3s.pop(g)))
                    if 0 <= s < ipc:
                        ps1, _ = h2s[s]
                        h2 = h2p.tile([H, nj], bf16)
                        if "tanh2" not in ablate or s < 4:
                            nc.scalar.activation(out=h2, in_=ps1, func=Tanh,
                                                 bias=B2)   # ACT
                        h2s[s] = (ps1, h2)
                for g, ps3 in [(g, p) for g, p in ps3s.items()]:
                    tanh3(g, ps3)
                    redq.append((g, ps3))
                for ent in redq:
                    reduce_group(*ent)

            if reps == 1:
                group_pass()
            else:
                # hardware loop: identical iterations, used for timing only
                with tc.For_i(0, reps):
                    group_pass()

            # --- reduce over the 128 j-offset partitions: out = ACC.T @ ones
            fin_t = psB.tile([H, G, NCH, OPAD], f32, tag="s3")
            fin = fin_t[:, 0, 0, 0:1]
            nc.tensor.matmul(fin, ACC, ONES, start=True, stop=True)
            yout = scrp.tile([ipc, 1], f32, tag="yout")
            nc.scalar.copy(yout, fin)
            nc.sync.dma_start(out=yparam[:, :], in_=yout)

    # populate .instr bytes for InstISA subclasses (custom-DVE ops); raw
    # Bass skips this Bacc pass and walrus then fails with "ISA wrong length"
    mybir.codegen_inst_isa_subclasses(nc)
    _legalize_waits(nc)
    return nc


_NC_CACHE = {}


def _get_nc(ipc, nj):
    key = (ipc, nj)
    if key not in _NC_CACHE:
        _NC_CACHE[key] = _build(ipc, nj)
    return _NC_CACHE[key]


def make_in_maps(x, W1, b1, W2, b2, W3, b3):
    """Per-core packed const blocks. Returns list of {"c": [H,CCOLS] f32}."""
    x = np.asarray(x, np.float32)
    W1 = np.asarray(W1, np.float32)
    hi = x @ W1[:F]          # [N, H]
    hj = x @ W1[F:]          # [N, H]
    hib = hi + np.asarray(b1, np.float32)[None, :]   # fold b1
    hj_t = np.ascontiguousarray(hj[:NJ].T)           # [H, nj]
    W3pad = np.zeros((H, OPAD), np.float32)
    W3pad[:, :O] = np.asarray(W3, np.float32)

    hjb = _bf16_pack(hj_t)
    tb = np.tanh(hj_t)
    w2b = _bf16_pack(np.asarray(W2, np.float32))
    w3b = _bf16_pack(W3pad)

    in_maps = []
    for c in range(NCORES):
        blk = np.empty((H, CCOLS), np.float32)
        hibc = hib[c * IPC:(c + 1) * IPC].T          # [H, IPC]
        blk[:, _HJB:_HIB] = hjb
        blk[:, _HIB:_TB] = hibc
        blk[:, _TB:_TA] = tb
        blk[:, _TA:_W2B] = np.tanh(hibc)
        blk[:, _W2B:_W3B] = w2b
        blk[:, _W3B:_B2] = w3b
        blk[:, _B2] = np.asarray(b2, np.float32)
        blk[:, _ONES] = 1.0
        in_maps.append({"c": blk})
    return in_maps


def kernel(x, W1, b1, W2, b2, W3, b3):
    b3 = np.asarray(b3, np.float32)
    if np.any(b3 != 0.0):
        # Never hit for this problem (spec fills b3 with zeros); exact
        # numpy fallback keeps the kernel correct for arbitrary inputs.
        return _numpy_ref(np.asarray(x, np.float32), np.asarray(W1, np.float32),
                          np.asarray(b1, np.float32), np.asarray(W2, np.float32),
                          np.asarray(b2, np.float32), np.asarray(W3, np.float32),
                          b3)

    in_maps = make_in_maps(x, W1, b1, W2, b2, W3, b3)
    nc = _get_nc(IPC, NJ)
    res = run_bass_kernel_spmd(nc, in_maps, list(range(NCORES)))
    out = np.concatenate(
        [res.results[c]["y"].reshape(IPC) for c in range(NCORES)]
    ).astype(np.float32)
    return out


def _numpy_ref(x, W1, b1, W2, b2, W3, b3):
    hi = x @ W1[:F]
    hj = x @ W1[F:]
    out = np.empty((N,), np.float32)
    for i in range(N):
        h = np.tanh(hi[i][None, :] + hj + b1[None, :])
        h = np.tanh(h @ W2 + b2[None, :])
        y = np.tanh(h @ W3 + b3[None, :])
        out[i] = y.sum()
    return out


# revision 20
# speedup vs baseline: 1.1348x; 1.0736x over previous
"""Trainium2 Bass kernel for nn_ConvPair (pairwise-MLP message passing).

Reference computation (N=1024 atoms, F=8 feats, H=128 hidden, O=3 out):
    hi = x @ W1[:F];  hj = x @ W1[F:]
    h  = tanh(hi[:,None,:] + hj[None,:,:] + b1)        # [N,N,H]
    h  = tanh(h @ W2 + b2)                             # [N,N,H]
    y  = tanh(h @ W3 + b3)                             # [N,N,O]
    out = y.sum(axis=(1,2))                            # [N]

Sharding: outer atom dim i split across 8 cores (128 i per core); the small
weights and the full atom table are replicated. No cross-core reduction.

The ACT (scalar) engine is the roofline: 2 tanh per (pair, hidden) element
= 262k columns/core at 1 col/cycle @1.2GHz. To get under that, tanh1 is
SPLIT between ACT and the otherwise-idle DVE:
  - j < AJ:   DVE broadcast-add (bf16, 4x mode), one big ACT tanh per group
  - j >= AJ:  exact tanh addition identity on DVE only, using
              host-precomputed tb = tanh(hj), ta = tanh(hi + b1):
                den = tb*ta_i + 1      (tensor_scalar 2-op, fp32, 2x)
                r   = 1/den            (reciprocal_approx_fast, 1 pass)
                h1  = (tb + ta_i) * r  (scalar_tensor_tensor, bf16 out)
All matmuls run in bf16 (1 cyc/row on PE + fast weight load). Final
accuracy ~3e-3 rel (gate 2e-2).

Per-core device pipeline, hidden-major [H=128 partitions, ...]:
  prep:  DVE  h1 pre-activations/identity-tanh for the NEXT group
  tanh1: ACT  one in-place tanh per group of G=16 i  [128, G*AJ]
  mm1:   PE   W2 stationary (bf16, FWL), h1 moving -> ps1 [128,1024] f32
  tanh2: ACT  tanh(ps1 + b2) -> h2 bf16 SBUF      bias = b2 column
  mm3:   PE   8x (h2 128-pair chunk stationary) @ W3pad -> ps3 [128,4]
  tanh3: ACT  one in-place tanh per group over ps3 [128, G*32]
  red:   DVE  free-axis reduce ps3 -> ACC[:, i]   (j-offset partials)
  final: PE   ACC.T @ ones -> per-i scalars, ACT copy, DMA out.

The PE stream is software-pipelined (mm1 of step k+1 emitted before mm3 of
step k) so the PE ping-pong never blocks ACT; mm3/tanh3/reduce of a group's
last step drain during the next group's big tanh1.

b3 is zeros for this problem (asserted; a numpy fallback handles the
hypothetical nonzero case exactly).

Wait-discipline: walrus's datapath codegen supports only one semaphore
wait per instruction; _legalize_waits hoists extras onto chained NoOps.
"""

import json

import numpy as np
from contextlib import ExitStack

import bass_rust
import concourse.bass as bass
import concourse.tile as tile
from concourse import mybir
from concourse.bass_utils import run_bass_kernel_spmd

f32 = mybir.dt.float32
bf16 = mybir.dt.bfloat16
Tanh = mybir.ActivationFunctionType.Tanh
Alu = mybir.AluOpType

N, F, H, O = 1024, 8, 128, 3
NCORES = 8
IPC = N // NCORES  # 128 atoms (i) per core
NJ = N             # full j dimension on every core
G = 16             # i's per group (tanh1/tanh3 batching)
NCH = NJ // H      # 8 stage-3 chunks of 128 pairs
OPAD = 4           # W3 padded 3 -> 4 cols (aligned psum writes; pad col = 0)
AJ = 608           # j-columns of tanh1 handled by ACT; rest by DVE identity
J2 = NJ - AJ

# fp32-column layout of the packed per-core constant block [H, CCOLS]
_HJB = 0                  # hj.T bf16-packed            [H, NJ/2]
_HIB = _HJB + NJ // 2     # (hi + b1).T fp32            [H, IPC]
_TB = _HIB + IPC          # tanh(hj).T fp32             [H, NJ]
_TA = _TB + NJ            # tanh(hi + b1).T fp32        [H, IPC]
_W2B = _TA + IPC          # W2 bf16-packed              [H, H/2]
_W3B = _W2B + H // 2      # W3pad bf16-packed           [H, OPAD/2]
_B2 = _W3B + OPAD // 2
_ONES = _B2 + 1
CCOLS = _ONES + 1


def _bf16_pack(a):
    """[P, C] f32 -> [P, C//2] f32 whose words hold bf16 pairs (RNE)."""
    u = np.ascontiguousarray(a, np.float32).view(np.uint32)
    r = ((u >> 16) & 1) + 0x7FFF
    h = ((u + r) >> 16).astype(np.uint32)          # bf16 bit patterns
    h = h.reshape(a.shape[0], -1, 2)
    return (h[:, :, 0] | (h[:, :, 1] << 16)).view(np.float32)


# TPB instructions have a single 8-byte events field: 2 sync commands max
# (walrus rejects more).  Queue-engine DMA ops handle their own sync.
_MULTIWAIT_OK = {"DMACopy", "TriggeredCopy", "Call", "ISA"}


def _legalize_waits(nc):
    """Hoist excess semaphore waits from datapath instructions onto chained
    NoOps (one wait each) so every instruction fits walrus's sync budget.
    Mutates the module in place (a JSON round-trip would drop the extra
    fields of InstCustomDveAnt and break codegen)."""
    counter = 0
    for fn in nc.m.functions:
        for blk in fn.blocks:
            insts = blk.instructions
            out = []
            for inst in insts:
                si = inst.sync_info
                waits = list(si.on_wait) if si is not None else []
                if (si is not None and len(waits) > 1
                        and str(inst.opcode) not in _MULTIWAIT_OK):
                    # keep zero waits on the instruction; one NoOp per wait
                    for w in waits:
                        counter += 1
                        out.append(mybir.InstNoOp(
                            name=f"W-hoist-{counter}",
                            engine=inst.engine,
                            debug=inst.debug,
                            ins=[],
                            outs=[],
                            sync_info=mybir.SyncInfo(on_wait=[w],
                                                     on_update=[]),
                        ))
                    si.on_wait = []
                out.append(inst)
            if len(out) != len(insts):
                del insts[:]
                insts.extend(out)
    return counter


def _build(ipc, nj, reps=1, aj=AJ, ablate=()):
    """Build the per-core Bass program (SPMD: same program, per-core data).

    reps > 1 repeats the main loop (recomputing identical results) via a
    hardware For_i and is used only for differential timing.
    ablate: subset of {"mm3","mm1","tanh1","tanh2","dve"} — drops those
    stages (WRONG OUTPUT; timing experiments only)."""
    assert nj % H == 0 and ipc % G == 0
    j2 = nj - aj
    ngrp = ipc // G

    nc = bass.Bass()
    cparam = nc.declare_dram_parameter("c", [H, CCOLS], f32, isOutput=False)
    yparam = nc.declare_dram_parameter("y", [ipc, 1], f32, isOutput=True)

    with tile.TileContext(nc) as tc:
        with ExitStack() as ctx:
            consts = ctx.enter_context(tc.tile_pool(name="consts", bufs=1))
            h1p = ctx.enter_context(tc.tile_pool(name="h1p", bufs=4))
            denp = ctx.enter_context(tc.tile_pool(name="denp", bufs=3))
            h2p = ctx.enter_context(tc.tile_pool(name="h2p", bufs=4))
            scrp = ctx.enter_context(tc.tile_pool(name="scrp", bufs=1))
            accp = ctx.enter_context(tc.tile_pool(name="accp", bufs=1))
            # PSUM budget (8 banks): ps1 3x2 + (ps3/warm/fin pool) 2x1
            psA = ctx.enter_context(tc.tile_pool(name="psA", bufs=3, space="PSUM"))
            psB = ctx.enter_context(tc.tile_pool(name="psB", bufs=2, space="PSUM"))

            C = consts.tile([H, CCOLS], f32)
            nc.sync.dma_start(out=C, in_=cparam[:, :])

            HJB = C[:, _HJB:_HIB].bitcast(bf16)       # [H, nj]    bf16
            HIB = C[:, _HIB:_TB]                      # [H, ipc]   f32
            TB = C[:, _TB:_TA]                        # [H, nj]    f32
            TA = C[:, _TA:_W2B]                       # [H, ipc]   f32
            W2B = C[:, _W2B:_W3B].bitcast(bf16)       # [H, H]     bf16
            W3B = C[:, _W3B:_B2].bitcast(bf16)        # [H, OPAD]  bf16
            B2 = C[:, _B2:_B2 + 1]                    # [H, 1]     f32
            ONES = C[:, _ONES:_ONES + 1]              # [H, 1]     f32
            TBD = TB[:, aj:]                          # DVE-share tanh(hj)

            ACC = accp.tile([H, ipc], f32)            # [j-offset, i] partials
            warm = scrp.tile([H, 1], f32, tag="warm")

            # --- warmups: let ACT, PE and DVE observe the const-DMA
            # semaphore (and load the tanh table) on single-wait
            # instructions, so no in-loop instruction waits on the DMA.
            nc.scalar.activation(out=warm, in_=B2, func=Tanh)
            warm_t = psB.tile([H, G, NCH, OPAD], f32, tag="s3")
            nc.tensor.matmul(warm_t[:1, 0, 0, :1], C[:, _B2:_B2 + 1],
                             C[:, _B2:_B2 + 1], start=True, stop=True)
            warmv = scrp.tile([H, 1], f32, tag="warmv")
            nc.vector.tensor_scalar_add(warmv, B2, 0.0)

            def prep_dve(i, h1s, dens):
                """DVE: exact identity-tanh for slot i's j >= aj columns."""
                h1 = h1p.tile([H, nj], bf16, tag="h1")
                if "dve" in ablate:
                    nc.vector.tensor_scalar_add(
                        h1[:, aj:], HJB[:, aj:], HIB[:, i:i + 1])
                    h1s[i] = h1
                    return
                den = denp.tile([H, j2], f32, tag="den")
                nc.vector.tensor_scalar(
                    den, TBD, TA[:, i:i + 1], 1.0, Alu.mult, Alu.add)
                nc.vector.reciprocal_approx_fast(out=den, in_=den)
                nc.vector.scalar_tensor_tensor(
                    out=h1[:, aj:], in0=TBD, scalar=TA[:, i:i + 1],
                    in1=den, op0=Alu.add, op1=Alu.mult)
                h1s[i], dens[i] = h1, den

            def tanh1(i, h1s):
                """ACT: tanh(hj + hib_i) for slot i's j < aj columns, via the
                free affine bias port."""
                if "tanh1" in ablate:
                    if i == 0:  # h1 ACT-share must be written at least once
                        nc.scalar.activation(out=h1s[i][:, :aj],
                                             in_=HJB[:, :aj], func=Tanh,
                                             bias=HIB[:, i:i + 1])
                    return
                nc.scalar.activation(out=h1s[i][:, :aj], in_=HJB[:, :aj],
                                     func=Tanh, bias=HIB[:, i:i + 1])

            def flush_mm3(p):
                """Emit the deferred stage-3 matmuls for one (group, k)."""
                ps3, k, h2 = p
                for c in range(1 if "mm3" in ablate else NCH):
                    nc.tensor.matmul(
                        ps3[:, k, c, :],
                        h2[:, c * H:(c + 1) * H],
                        W3B,
                        start=True, stop=True,
                    )

            def tanh3(g, ps3):
                nc.scalar.activation(out=ps3[:, :, :, :], in_=ps3[:, :, :, :],
                                     func=Tanh)

            def reduce_group(g, ps3):
                nc.vector.tensor_reduce(
                    out=ACC[:, g * G:(g + 1) * G],
                    in_=ps3.rearrange("p g c o -> p g (c o)"),
                    axis=mybir.AxisListType.X,
                    op=mybir.AluOpType.add,
                )

            def group_pass():
                """Deep software pipeline over 128 slots: at slot s, DVE
                preps slot s+4, ACT tanh1's slot s+2, PE runs mm1(s+1) and
                mm3(s-2), ACT tanh2's slot s.  Every consumer's input is
                ready >=1 slot early, so no engine blocks on another within
                a slot."""
                h1s, dens, h2s, ps3s = {}, {}, {}, {}
                t3q = []              # groups whose last mm3 was emitted
                redq = []             # groups past tanh3, awaiting reduce
                for s in range(-4, ipc + 4):
                    ip = s + 4
                    if 0 <= ip < ipc:
                        prep_dve(ip, h1s, dens)         # DVE
                    if redq:
                        reduce_group(*redq.pop(0))      # DVE, after prep
                    im = s + 1
                    if 0 <= im < ipc:
                        h1 = h1s.pop(im)
                        ps1 = psA.tile([H, nj], f32)
                        if "mm1" not in ablate or im < 3:
                            for t in range(nj // 512):
                                nc.tensor.matmul(
                                    ps1[:, t * 512:(t + 1) * 512],
                                    W2B, h1[:, t * 512:(t + 1) * 512],
                                    start=True, stop=True)
                        h2s[im] = (ps1, None)
                    i3 = s - 2
                    if 0 <= i3 < ipc:
                        g3, k3 = divmod(i3, G)
                        if k3 == 0:
                            ps3_t = psB.tile([H, G, NCH, OPAD], f32,
                                             tag="s3")
                            ps3s[g3] = ps3_t
                        flush_mm3((ps3s[g3], k3, h2s.pop(i3)[1]))   # PE
                        if k3 == G - 1:
                            t3q.append(g3)
                    it1 = s + 2
                    if 0 <= it1 < ipc:
                        tanh1(it1, h1s)                 # ACT
                    if t3q and t3q[0] * G + G - 1 <= s - 3:
                        g = t3q.pop(0)
                        tanh3(g, ps3s[g])               # ACT
                        redq.append((g, ps3s.pop(g)))
                    if 0 <= s < ipc:
                        ps1, _ = h2s[s]
                        h2 = h2p.tile([H, nj], bf16)
                        if "tanh2" not in ablate or s < 4:
                            nc.scalar.activation(out=h2, in_=ps1, func=Tanh,
                                                 bias=B2)   # ACT
                        h2s[s] = (ps1, h2)
                for g, ps3 in [(g, p) for g, p in ps3s.items()]:
                    tanh3(g, ps3)
                    redq.append((g, ps3))
                for ent in redq:
                    reduce_group(*ent)

            if reps == 1:
                group_pass()
            else:
                # hardware loop: identical iterations, used for timing only
                with tc.For_i(0, reps):
                    group_pass()

            # --- reduce over the 128 j-offset partitions: out = ACC.T @ ones
            fin_t = psB.tile([H, G, NCH, OPAD], f32, tag="s3")
            fin = fin_t[:, 0, 0, 0:1]
            nc.tensor.matmul(fin, ACC, ONES, start=True, stop=True)
            yout = scrp.tile([ipc, 1], f32, tag="yout")
            nc.scalar.copy(yout, fin)
            nc.sync.dma_start(out=yparam[:, :], in_=yout)

    # populate .instr bytes for InstISA subclasses (custom-DVE ops); raw
    # Bass skips this Bacc pass and walrus then fails with "ISA wrong length"
    mybir.codegen_inst_isa_subclasses(nc)
    _legalize_waits(nc)
    return nc


_NC_CACHE = {}


def _get_nc(ipc, nj):
    key = (ipc, nj)
    if key not in _NC_CACHE:
        _NC_CACHE[key] = _build(ipc, nj)
    return _NC_CACHE[key]


def make_in_maps(x, W1, b1, W2, b2, W3, b3):
    """Per-core packed const blocks. Returns list of {"c": [H,CCOLS] f32}."""
    x = np.asarray(x, np.float32)
    W1 = np.asarray(W1, np.float32)
    hi = x @ W1[:F]          # [N, H]
    hj = x @ W1[F:]          # [N, H]
    hib = hi + np.asarray(b1, np.float32)[None, :]   # fold b1
    hj_t = np.ascontiguousarray(hj[:NJ].T)           # [H, nj]
    W3pad = np.zeros((H, OPAD), np.float32)
    W3pad[:, :O] = np.asarray(W3, np.float32)

    hjb = _bf16_pack(hj_t)
    tb = np.tanh(hj_t)
    w2b = _bf16_pack(np.asarray(W2, np.float32))
    w3b = _bf16_pack(W3pad)

    in_maps = []
    for c in range(NCORES):
        blk = np.empty((H, CCOLS), np.float32)
        hibc = hib[c * IPC:(c + 1) * IPC].T          # [H, IPC]
        blk[:, _HJB:_HIB] = hjb
        blk[:, _HIB:_TB] = hibc
        blk[:, _TB:_TA] = tb
        blk[:, _TA:_W2B] = np.tanh(hibc)
        blk[:, _W2B:_W3B] = w2b
        blk[:, _W3B:_B2] = w3b
        blk[:, _B2] = np.asarray(b2, np.float32)
        blk[:, _ONES] = 1.0
        in_maps.append({"c": blk})
    return in_maps


def kernel(x, W1, b1, W2, b2, W3, b3):
    b3 = np.asarray(b3, np.float32)
    if np.any(b3 != 0.0):
        # Never hit for this problem (spec fills b3 with zeros); exact
        # numpy fallback keeps the kernel correct for arbitrary inputs.
        return _numpy_ref(np.asarray(x, np.float32), np.asarray(W1, np.float32),
                          np.asarray(b1, np.float32), np.asarray(W2, np.float32),
                          np.asarray(b2, np.float32), np.asarray(W3, np.float32),
                          b3)

    in_maps = make_in_maps(x, W1, b1, W2, b2, W3, b3)
    nc = _get_nc(IPC, NJ)
    res = run_bass_kernel_spmd(nc, in_maps, list(range(NCORES)))
    out = np.concatenate(
        [res.results[c]["y"].reshape(IPC) for c in range(NCORES)]
    ).astype(np.float32)
    return out


def _numpy_ref(x, W1, b1, W2, b2, W3, b3):
    hi = x @ W1[:F]
    hj = x @ W1[F:]
    out = np.empty((N,), np.float32)
    for i in range(N):
        h = np.tanh(hi[i][None, :] + hj + b1[None, :])
        h = np.tanh(h @ W2 + b2[None, :])
        y = np.tanh(h @ W3 + b3[None, :])
        out[i] = y.sum()
    return out
